# revision 14
# baseline (speedup 1.0000x reference)
"""Trainium2 Bass kernel for masked multi-head attention (returns out AND attn).

Problem: B=4, N=3000 (120 frames x 25), D=512, H=8, DH=64.
  q/k = x@W+b per head; v = relu(x@Wv+bv)
  scores = q k^T / 8, masked so tokens can't attend within their own frame
  (except self), softmax, out = attn @ v.  Returns (out[B,N,512], attn[B,H,N,N]).

Sharding: 8 cores = (batch b = core//2) x (head-half hp = core%2, 4 heads each).
No cross-core communication.

Per-core plan ("dup-exp"): score matrices are computed on the PE in BOTH
orientations — S[q,k] for the attention output (contiguous HBM rows) and
S^T[k,q] for the P.V matmul (PE contracts over the partition dim, so P.V
needs k on partitions).  exp runs on the scalar engine for both (an exp
costs the same as the copy it replaces).  The in-frame mask is applied by
adding -1e5 to the masked 125x125 block on PSUM before exp (exp underflows
to exactly 0).  Row-sums come free from a ones-column appended to V.
"""

import contextlib
import ctypes
import sys
import types

import numpy as np

B, N, D, H, DH = 4, 3000, 512, 8, 64
J, F = 25, 120
QB = 125          # query/key tile (24 tiles; 125 = 5 frames exactly)
NT = N // QB      # 24
HPC = 4           # heads per core
NEGM = -1.0e5

_SO_PATH = "/opt/axon/libaxon_pjrt.so"


def _install_profile_hook():
    if "antenv.axon_hooks" in sys.modules:
        return
    try:
        lib = ctypes.CDLL(_SO_PATH)
        lib.axon_start_nrt_profile.argtypes = [
            ctypes.POINTER(ctypes.c_int64),
            ctypes.c_size_t,
        ]
        lib.axon_start_nrt_profile.restype = ctypes.c_int64
        lib.axon_stop_nrt_profile.argtypes = [ctypes.c_char_p]
        lib.axon_stop_nrt_profile.restype = ctypes.c_int64
    except OSError:
        return

    @contextlib.contextmanager
    def _hook(output_dir, device_ids):
        import jax

        jax.devices()
        if device_ids:
            ids = (ctypes.c_int64 * len(device_ids))(*device_ids)
            rc = lib.axon_start_nrt_profile(ids, len(device_ids))
        else:
            rc = lib.axon_start_nrt_profile(None, 0)
        if rc != 0:
            raise RuntimeError(f"axon_start_nrt_profile rc={rc}")
        try:
            yield
        finally:
            n = lib.axon_stop_nrt_profile(str(output_dir).encode())
            print(f"profile: {n} file(s) written to {output_dir}")

    mod = types.ModuleType("antenv.axon_hooks")
    mod.get_axon_ntff_profile_hook = lambda: _hook
    mod.set_axon_ntff_profile_hook = lambda h: None
    sys.modules["antenv.axon_hooks"] = mod


def _fix_multiwait(nc):
    """This walrus build accepts one sync wait per instruction; split any
    multi-wait instruction into single-wait EventSemaphore prefixes."""
    from concourse import mybir

    for fn in nc.m.functions:
        for bb in fn.blocks:
            new_list = []
            changed = False
            for inst in bb.instructions:
                si = getattr(inst, "sync_info", None)
                if si is not None and si.on_wait and len(si.on_wait) > 1:
                    waits = list(si.on_wait)
                    for j, w in enumerate(waits[:-1]):
                        new_list.append(
                            mybir.InstEventSemaphore(
                                name=f"{inst.name}-wsplit{j}",
                                engine=inst.engine,
                                ins=[],
                                outs=[],
                                sync_info=mybir.SyncInfo(on_wait=[w], on_update=[]),
                            )
                        )
                    si.on_wait = [waits[-1]]
                    changed = True
                new_list.append(inst)
            if changed:
                bb.instructions[:] = new_list
    return nc


def _build_bass(debug=False):
    import concourse.bass as bass
    import concourse.tile as tile
    from concourse import mybir

    f32 = mybir.dt.float32
    EXP = mybir.ActivationFunctionType.Exp

    nc = bass.Bass()
    dbg = {}
    if debug:
        dbg["qt0"] = nc.dram_tensor("dbg_qt0", [128, N], f32, kind="ExternalOutput")
        dbg["kt0"] = nc.dram_tensor("dbg_kt0", [128, N], f32, kind="ExternalOutput")
        dbg["v0"] = nc.dram_tensor("dbg_v0", [QB, NT * 65], f32, kind="ExternalOutput")
        dbg["outT0"] = nc.dram_tensor("dbg_outT0", [65, N], f32, kind="ExternalOutput")
        dbg["recipT0"] = nc.dram_tensor("dbg_recipT0", [QB, NT], f32, kind="ExternalOutput")
        dbg["xt"] = nc.dram_tensor("dbg_xt", [128, 4 * N], f32, kind="ExternalOutput")
    x_d = nc.dram_tensor("x", [N, D], f32, kind="ExternalInput")
    wq_d = nc.dram_tensor("wq", [D, 256], f32, kind="ExternalInput")
    wk_d = nc.dram_tensor("wk", [D, 256], f32, kind="ExternalInput")
    wv_d = nc.dram_tensor("wv", [D, 256], f32, kind="ExternalInput")
    bq_d = nc.dram_tensor("bq", [256], f32, kind="ExternalInput")
    bk_d = nc.dram_tensor("bk", [256], f32, kind="ExternalInput")
    bv_d = nc.dram_tensor("bv", [256], f32, kind="ExternalInput")
    mask_d = nc.dram_tensor("maskadd", [QB, QB], f32, kind="ExternalInput")
    id_d = nc.dram_tensor("ident", [128, 128], f32, kind="ExternalInput")
    attn_o = nc.dram_tensor("attn_o", [HPC, N, N], f32, kind="ExternalOutput")
    out_o = nc.dram_tensor("out_o", [N, 256], f32, kind="ExternalOutput")

    with tile.TileContext(nc) as tc:
        with contextlib.ExitStack() as ctx:
            cst = ctx.enter_context(tc.tile_pool(name="cst", bufs=1))
            qkt = ctx.enter_context(tc.tile_pool(name="qkt", bufs=1))
            vpool = ctx.enter_context(tc.tile_pool(name="vpool", bufs=1))
            # Shared PSUM pool: tag "a" 2 banks x2, "b" 2 banks x1,
            # "ot" 1 bank x2 -> 8 banks exactly.
            ps = ctx.enter_context(tc.tile_pool(name="ps", bufs=2, space="PSUM"))
            psb = ctx.enter_context(tc.tile_pool(name="psb", bufs=1, space="PSUM"))

            phase01 = ctx.enter_context(contextlib.ExitStack())
            wpool = phase01.enter_context(tc.tile_pool(name="wpool", bufs=1))
            xtp = phase01.enter_context(tc.tile_pool(name="xtp", bufs=1))
            xin = phase01.enter_context(tc.tile_pool(name="xin", bufs=3))

            # ---- constants ----
            mask_sb = cst.tile([QB, QB], f32, tag="mask")
            nc.sync.dma_start(out=mask_sb[:], in_=mask_d[:])
            id_sb = cst.tile([128, 128], f32, tag="ident")
            nc.sync.dma_start(out=id_sb[:], in_=id_d[:])
            ones_sb = cst.tile([1, QB], f32, tag="ones")
            nc.vector.memset(ones_sb[:], 1.0)
            bqp = cst.tile([128, 2], f32, tag="bqp")
            bkp = cst.tile([128, 2], f32, tag="bkp")
            nc.sync.dma_start(out=bqp[:], in_=bq_d.rearrange("(p d) -> d p", p=2))
            nc.sync.dma_start(out=bkp[:], in_=bk_d.rearrange("(p d) -> d p", p=2))
            bvr = cst.tile([1, 256], f32, tag="bvr")
            nc.sync.dma_start(out=bvr[:], in_=bv_d.rearrange("(o c) -> o c", o=1))

            # ---- weights ----
            w_sb = {}
            for nm, wd in (("q", wq_d), ("k", wk_d), ("v", wv_d)):
                w = wpool.tile([128, 4 * 256], f32, tag=f"w{nm}")
                for di in range(4):
                    nc.sync.dma_start(
                        out=w[:, di * 256 : (di + 1) * 256],
                        in_=wd[di * 128 : (di + 1) * 128, :],
                    )
                w_sb[nm] = w

            # ---- x -> xT  (xT[:, di*N + t*QB + j] = x[t*QB + j, di*128 + p]) ----
            xT = xtp.tile([128, 4 * N], f32, tag="xT")
            for t in range(NT):
                xt_in = xin.tile([QB, D], f32)
                nc.sync.dma_start(out=xt_in[:], in_=x_d[t * QB : (t + 1) * QB, :])
                for di in range(4):
                    pt = ps.tile([128, QB], f32, tag="ot")
                    nc.tensor.transpose(
                        pt[:], xt_in[:, di * 128 : (di + 1) * 128], id_sb[:QB, :QB]
                    )
                    nc.vector.tensor_copy(
                        xT[:, di * N + t * QB : di * N + (t + 1) * QB], pt[:]
                    )

            # ---- projections: QT/KT per pair [128, N] (two heads stacked) ----
            qt_sb = []
            kt_sb = []
            for p in range(2):
                qt = qkt.tile([128, N], f32, tag=f"qt{p}")
                kt = qkt.tile([128, N], f32, tag=f"kt{p}")
                for c in range(6):
                    cs = slice(c * 500, (c + 1) * 500)
                    for dst, w, bias in ((qt, w_sb["q"], bqp), (kt, w_sb["k"], bkp)):
                        psq = ps.tile([128, 500], f32, tag="a")
                        for di in range(4):
                            nc.tensor.matmul(
                                psq[:],
                                w[:, di * 256 + p * 128 : di * 256 + (p + 1) * 128],
                                xT[:, di * N + c * 500 : di * N + (c + 1) * 500],
                                start=(di == 0),
                                stop=(di == 3),
                            )
                        nc.vector.tensor_scalar_add(dst[:, cs], psq[:], bias[:, p : p + 1])
                qt_sb.append(qt)
                kt_sb.append(kt)

            # ---- V per head [QB, 24*65]; col 64 of each 65-block stays 1.0 ----
            v_sb = []
            for h in range(HPC):
                v = vpool.tile([QB, NT * 65], f32, tag=f"v{h}")
                nc.vector.memset(v[:], 1.0)
                v_sb.append(v)
            for p in range(2):
                for t in range(NT):
                    psv = ps.tile([QB, 128], f32, tag="ot")
                    for di in range(4):
                        nc.tensor.matmul(
                            psv[:],
                            xT[:, di * N + t * QB : di * N + (t + 1) * QB],
                            w_sb["v"][:, di * 256 + p * 128 : di * 256 + (p + 1) * 128],
                            start=(di == 0),
                            stop=False,
                        )
                    nc.tensor.matmul(
                        psv[:],
                        ones_sb[:1, :QB],
                        bvr[:1, p * 128 : (p + 1) * 128],
                        start=False,
                        stop=True,
                    )
                    for hh in range(2):
                        h = p * 2 + hh
                        nc.vector.tensor_scalar_max(
                            v_sb[h][:, t * 65 : t * 65 + 64],
                            psv[:, hh * 64 : (hh + 1) * 64],
                            0.0,
                        )

            if debug:
                nc.sync.dma_start(out=dbg["xt"][:], in_=xT[:])
            phase01.close()

            phase2 = ctx.enter_context(contextlib.ExitStack())
            otp = phase2.enter_context(tc.tile_pool(name="otp", bufs=2))
            rcp = phase2.enter_context(tc.tile_pool(name="rcp", bufs=2))
            etp = phase2.enter_context(tc.tile_pool(name="etp", bufs=3))
            attp = phase2.enter_context(tc.tile_pool(name="attp", bufs=2))
            obp = phase2.enter_context(tc.tile_pool(name="obp", bufs=3))
            osg = phase2.enter_context(tc.tile_pool(name="osg", bufs=1))
            ostage = osg.tile([QB, NT * 256], f32, tag="ostage")

            # ---- per head: PV phase then attn phase ----
            for h in range(HPC):
                p, hb = h // 2, (h % 2) * 64
                qt, kt, v = qt_sb[p], kt_sb[p], v_sb[h]

                # PV: S^T[k,q] in q-thirds; exp; accumulate [V|1]^T E^T
                outT = otp.tile([65, N], f32)
                for g in range(3):
                    bps = psb.tile([65, 1024], f32, tag="b")
                    for t in range(NT):
                        aps = ps.tile([QB, 1024], f32, tag="a")
                        for c in range(2):
                            nc.tensor.matmul(
                                aps[:, c * 512 : c * 512 + 500],
                                kt[hb : hb + 64, t * QB : (t + 1) * QB],
                                qt[hb : hb + 64, g * 1000 + c * 500 : g * 1000 + (c + 1) * 500],
                            )
                        if t // 8 == g:
                            off = t * QB - g * 1000
                            ccol = off if off < 500 else off + 12
                            nc.vector.tensor_add(
                                aps[:, ccol : ccol + QB], aps[:, ccol : ccol + QB], mask_sb[:]
                            )
                        et = etp.tile([QB, 1000], f32)
                        nc.scalar.activation(
                            et[:].rearrange("p (c w) -> p c w", w=500),
                            aps[:].rearrange("p (c w) -> p c w", w=512)[:, :, 0:500],
                            EXP,
                            scale=0.125,
                        )
                        for c in range(2):
                            nc.tensor.matmul(
                                bps[:, c * 512 : c * 512 + 500],
                                v[:, t * 65 : (t + 1) * 65],
                                et[:, c * 500 : (c + 1) * 500],
                                start=(t == 0),
                                stop=(t == NT - 1),
                            )
                    nc.vector.tensor_copy(
                        outT[:, g * 1000 : (g + 1) * 1000].rearrange("p (c w) -> p c w", w=500),
                        bps[:].rearrange("p (c w) -> p c w", w=512)[:, :, 0:500],
                    )

                # row 64 = row-sums -> reciprocals
                nc.vector.reciprocal(outT[64:65, :], outT[64:65, :])

                # transpose out^T blocks; scale rows by recip; stash recipT
                recipT = rcp.tile([QB, NT], f32)
                for t in range(NT):
                    ot = ps.tile([QB, 65], f32, tag="ot")
                    nc.tensor.transpose(
                        ot[:], outT[:, t * QB : (t + 1) * QB], id_sb[:65, :65]
                    )
                    ob = obp.tile([QB, 64], f32)
                    nc.vector.tensor_copy(ob[:], ot[:, 0:64])
                    nc.vector.tensor_copy(recipT[:, t : t + 1], ot[:, 64:65])
                    nc.vector.tensor_scalar_mul(
                        ostage[:, t * 256 + h * 64 : t * 256 + (h + 1) * 64],
                        ob[:],
                        recipT[:, t : t + 1],
                    )

                if debug and h == 0:
                    nc.sync.dma_start(out=dbg["qt0"][:], in_=qt_sb[0][:])
                    nc.sync.dma_start(out=dbg["kt0"][:], in_=kt_sb[0][:])
                    nc.sync.dma_start(out=dbg["v0"][:], in_=v_sb[0][:])
                    nc.sync.dma_start(out=dbg["outT0"][:], in_=outT[:])
                    nc.sync.dma_start(out=dbg["recipT0"][:], in_=recipT[:])

                # attn: S[q,k] per q-tile (in k-thirds), exp, normalize, DMA out
                for t in range(NT):
                    att = attp.tile([QB, N], f32)
                    for g in range(3):
                        sa = ps.tile([QB, 1024], f32, tag="a")
                        for c in range(2):
                            nc.tensor.matmul(
                                sa[:, c * 512 : c * 512 + 500],
                                qt[hb : hb + 64, t * QB : (t + 1) * QB],
                                kt[hb : hb + 64, g * 1000 + c * 500 : g * 1000 + (c + 1) * 500],
                            )
                        if t // 8 == g:
                            off = t * QB - g * 1000
                            ccol = off if off < 500 else off + 12
                            nc.vector.tensor_add(
                                sa[:, ccol : ccol + QB], sa[:, ccol : ccol + QB], mask_sb[:]
                            )
                        nc.scalar.activation(
                            att[:, g * 1000 : (g + 1) * 1000].rearrange("p (c w) -> p c w", w=500),
                            sa[:].rearrange("p (c w) -> p c w", w=512)[:, :, 0:500],
                            EXP,
                            scale=0.125,
                        )
                    nc.vector.tensor_scalar_mul(att[:], att[:], recipT[:, t : t + 1])
                    nc.sync.dma_start(
                        out=attn_o[h, t * QB : (t + 1) * QB, :], in_=att[:]
                    )

            # ---- final out DMA ----
            nc.sync.dma_start(
                out=out_o.rearrange("(t p) c -> p t c", p=QB),
                in_=ostage[:].rearrange("p (t c) -> p t c", c=256),
            )

    _fix_multiwait(nc)
    return nc


_CACHE = {}
TRACE = False
TRACE_KWARGS = {}
LAST_RESULT = [None]


def _get_nc():
    if "nc" not in _CACHE:
        _CACHE["nc"] = _build_bass()
    return _CACHE["nc"]


def _mask_np():
    block = np.kron(np.eye(5, dtype=np.float32), np.ones((J, J), np.float32))
    return (NEGM * (block - np.eye(QB, dtype=np.float32))).astype(np.float32)


def kernel(x, Wq, bq, Wk, bk, Wv, bv):
    _install_profile_hook()
    from concourse.bass_utils import run_bass_kernel_spmd
    from concourse import bass_utils

    bass_utils.upload_artifacts = lambda tmpdir: f"local://{tmpdir}"

    x = np.asarray(x, dtype=np.float32)
    Wq, Wk, Wv = (np.asarray(a, np.float32) for a in (Wq, Wk, Wv))
    bq, bk, bv = (np.asarray(a, np.float32) for a in (bq, bk, bv))

    mask = _mask_np()
    ident = np.eye(128, dtype=np.float32)

    in_maps = []
    for c in range(8):
        b, hp = c // 2, c % 2
        cols = slice(hp * 256, (hp + 1) * 256)
        in_maps.append(
            {
                "x": np.ascontiguousarray(x[b]),
                "wq": np.ascontiguousarray(Wq[:, cols]),
                "wk": np.ascontiguousarray(Wk[:, cols]),
                "wv": np.ascontiguousarray(Wv[:, cols]),
                "bq": np.ascontiguousarray(bq[cols]),
                "bk": np.ascontiguousarray(bk[cols]),
                "bv": np.ascontiguousarray(bv[cols]),
                "maskadd": mask,
                "ident": ident,
            }
        )

    nc = _get_nc()
    res = run_bass_kernel_spmd(
        nc, in_maps, list(range(8)), trace=TRACE, **TRACE_KWARGS
    )
    LAST_RESULT[0] = res

    out = np.empty((B, N, 512), np.float32)
    attn = np.empty((B, H, N, N), np.float32)
    for c in range(8):
        b, hp = c // 2, c % 2
        out[b, :, hp * 256 : (hp + 1) * 256] = res.results[c]["out_o"]
        attn[b, hp * 4 : (hp + 1) * 4] = res.results[c]["attn_o"]
    return out, attn


# revision 15
# speedup vs baseline: 1.3358x; 1.3358x over previous
"""Trainium2 Bass kernel for masked multi-head attention (returns out AND attn).

Problem: B=4, N=3000 (120 frames x 25), D=512, H=8, DH=64.
  q/k = x@W+b per head; v = relu(x@Wv+bv)
  scores = q k^T / 8, masked so tokens can't attend within their own frame
  (except self), softmax, out = attn @ v.  Returns (out[B,N,512], attn[B,H,N,N]).

Sharding: 8 cores = (batch b = core//2) x (head-half hp = core%2, 4 heads each).
No cross-core communication.

Per-core plan ("dup-exp"): score matrices are computed on the PE in BOTH
orientations — S[q,k] for the attention output (contiguous HBM rows) and
S^T[k,q] for the P.V matmul (PE contracts over the partition dim, so P.V
needs k on partitions).  exp runs on the scalar engine for both (an exp
costs the same as the copy it replaces).  The in-frame mask is applied by
adding -1e5 to the masked 125x125 block on PSUM before exp (exp underflows
to exactly 0).  Row-sums come free from a ones-column appended to V.
"""

import contextlib
import ctypes
import sys
import types

import numpy as np

B, N, D, H, DH = 4, 3000, 512, 8, 64
J, F = 25, 120
QB = 125          # query/key tile (24 tiles; 125 = 5 frames exactly)
NT = N // QB      # 24
HPC = 4           # heads per core
NEGM = -1.0e5

_SO_PATH = "/opt/axon/libaxon_pjrt.so"


def _install_profile_hook():
    if "antenv.axon_hooks" in sys.modules:
        return
    try:
        lib = ctypes.CDLL(_SO_PATH)
        lib.axon_start_nrt_profile.argtypes = [
            ctypes.POINTER(ctypes.c_int64),
            ctypes.c_size_t,
        ]
        lib.axon_start_nrt_profile.restype = ctypes.c_int64
        lib.axon_stop_nrt_profile.argtypes = [ctypes.c_char_p]
        lib.axon_stop_nrt_profile.restype = ctypes.c_int64
    except OSError:
        return

    @contextlib.contextmanager
    def _hook(output_dir, device_ids):
        import jax

        jax.devices()
        if device_ids:
            ids = (ctypes.c_int64 * len(device_ids))(*device_ids)
            rc = lib.axon_start_nrt_profile(ids, len(device_ids))
        else:
            rc = lib.axon_start_nrt_profile(None, 0)
        if rc != 0:
            raise RuntimeError(f"axon_start_nrt_profile rc={rc}")
        try:
            yield
        finally:
            n = lib.axon_stop_nrt_profile(str(output_dir).encode())
            print(f"profile: {n} file(s) written to {output_dir}")

    mod = types.ModuleType("antenv.axon_hooks")
    mod.get_axon_ntff_profile_hook = lambda: _hook
    mod.set_axon_ntff_profile_hook = lambda h: None
    sys.modules["antenv.axon_hooks"] = mod


def _fix_multiwait(nc):
    """This walrus build accepts one sync wait per instruction; split any
    multi-wait instruction into single-wait EventSemaphore prefixes."""
    from concourse import mybir

    for fn in nc.m.functions:
        for bb in fn.blocks:
            new_list = []
            changed = False
            for inst in bb.instructions:
                si = getattr(inst, "sync_info", None)
                if si is not None and si.on_wait and len(si.on_wait) > 1:
                    waits = list(si.on_wait)
                    for j, w in enumerate(waits[:-1]):
                        new_list.append(
                            mybir.InstEventSemaphore(
                                name=f"{inst.name}-wsplit{j}",
                                engine=inst.engine,
                                ins=[],
                                outs=[],
                                sync_info=mybir.SyncInfo(on_wait=[w], on_update=[]),
                            )
                        )
                    si.on_wait = [waits[-1]]
                    changed = True
                new_list.append(inst)
            if changed:
                bb.instructions[:] = new_list
    return nc


def _build_bass(debug=False):
    import concourse.bass as bass
    import concourse.tile as tile
    from concourse import mybir

    f32 = mybir.dt.float32
    bf16 = mybir.dt.bfloat16
    EXP = mybir.ActivationFunctionType.Exp

    nc = bass.Bass()
    dbg = {}
    if debug:
        dbg["qt0"] = nc.dram_tensor("dbg_qt0", [128, N], f32, kind="ExternalOutput")
        dbg["kt0"] = nc.dram_tensor("dbg_kt0", [128, N], f32, kind="ExternalOutput")
        dbg["v0"] = nc.dram_tensor("dbg_v0", [QB, NT * 65], f32, kind="ExternalOutput")
        dbg["outT0"] = nc.dram_tensor("dbg_outT0", [65, N], f32, kind="ExternalOutput")
        dbg["recipT0"] = nc.dram_tensor("dbg_recipT0", [QB, NT], f32, kind="ExternalOutput")
        dbg["xt"] = nc.dram_tensor("dbg_xt", [128, 4 * N], f32, kind="ExternalOutput")
    x_d = nc.dram_tensor("x", [N, D], f32, kind="ExternalInput")
    wq_d = nc.dram_tensor("wq", [D, 256], f32, kind="ExternalInput")
    wk_d = nc.dram_tensor("wk", [D, 256], f32, kind="ExternalInput")
    wv_d = nc.dram_tensor("wv", [D, 256], f32, kind="ExternalInput")
    bq_d = nc.dram_tensor("bq", [256], f32, kind="ExternalInput")
    bk_d = nc.dram_tensor("bk", [256], f32, kind="ExternalInput")
    bv_d = nc.dram_tensor("bv", [256], f32, kind="ExternalInput")
    mask_d = nc.dram_tensor("maskadd", [QB, QB], f32, kind="ExternalInput")
    id_d = nc.dram_tensor("ident", [128, 128], f32, kind="ExternalInput")
    attn_o = nc.dram_tensor("attn_o", [HPC, N, N], f32, kind="ExternalOutput")
    out_o = nc.dram_tensor("out_o", [N, 256], f32, kind="ExternalOutput")

    with tile.TileContext(nc) as tc:
        with contextlib.ExitStack() as ctx:
            cst = ctx.enter_context(tc.tile_pool(name="cst", bufs=1))
            qkt = ctx.enter_context(tc.tile_pool(name="qkt", bufs=1))
            vpool = ctx.enter_context(tc.tile_pool(name="vpool", bufs=1))
            # Shared PSUM pool: tag "a" 2 banks x2, "b" 2 banks x1,
            # "ot" 1 bank x2 -> 8 banks exactly.
            ps = ctx.enter_context(tc.tile_pool(name="ps", bufs=2, space="PSUM"))
            psb = ctx.enter_context(tc.tile_pool(name="psb", bufs=1, space="PSUM"))

            phase01 = ctx.enter_context(contextlib.ExitStack())
            wpool = phase01.enter_context(tc.tile_pool(name="wpool", bufs=1))
            xtp = phase01.enter_context(tc.tile_pool(name="xtp", bufs=1))
            xin = phase01.enter_context(tc.tile_pool(name="xin", bufs=3))

            # ---- constants ----
            mask_sb = cst.tile([QB, QB], f32, tag="mask")
            nc.sync.dma_start(out=mask_sb[:], in_=mask_d[:])
            id_sb = cst.tile([128, 128], f32, tag="ident")
            nc.sync.dma_start(out=id_sb[:], in_=id_d[:])
            ones_sb = cst.tile([1, QB], bf16, tag="ones")
            nc.vector.memset(ones_sb[:], 1.0)
            bqp = cst.tile([128, 2], f32, tag="bqp")
            bkp = cst.tile([128, 2], f32, tag="bkp")
            nc.sync.dma_start(out=bqp[:], in_=bq_d.rearrange("(p d) -> d p", p=2))
            nc.sync.dma_start(out=bkp[:], in_=bk_d.rearrange("(p d) -> d p", p=2))
            bvr = cst.tile([1, 256], bf16, tag="bvr")
            nc.gpsimd.dma_start(out=bvr[:], in_=bv_d.rearrange("(o c) -> o c", o=1))

            # ---- weights ----
            w_sb = {}
            for nm, wd in (("q", wq_d), ("k", wk_d), ("v", wv_d)):
                w = wpool.tile([128, 4 * 256], bf16, tag=f"w{nm}")
                for di in range(4):
                    nc.gpsimd.dma_start(
                        out=w[:, di * 256 : (di + 1) * 256],
                        in_=wd[di * 128 : (di + 1) * 128, :],
                    )
                w_sb[nm] = w

            # ---- x -> xT  (xT[:, di*N + t*QB + j] = x[t*QB + j, di*128 + p]) ----
            xT = xtp.tile([128, 4 * N], bf16, tag="xT")
            for t in range(NT):
                xt_in = xin.tile([QB, D], f32)
                nc.sync.dma_start(out=xt_in[:], in_=x_d[t * QB : (t + 1) * QB, :])
                for di in range(4):
                    pt = ps.tile([128, QB], f32, tag="ot")
                    nc.tensor.transpose(
                        pt[:], xt_in[:, di * 128 : (di + 1) * 128], id_sb[:QB, :QB]
                    )
                    nc.vector.tensor_copy(
                        xT[:, di * N + t * QB : di * N + (t + 1) * QB], pt[:]
                    )

            # ---- projections: QT/KT per pair [128, N] (two heads stacked) ----
            qt_sb = []
            kt_sb = []
            for p in range(2):
                qt = qkt.tile([128, N], bf16, tag=f"qt{p}")
                kt = qkt.tile([128, N], bf16, tag=f"kt{p}")
                for c in range(6):
                    cs = slice(c * 500, (c + 1) * 500)
                    for dst, w, bias in ((qt, w_sb["q"], bqp), (kt, w_sb["k"], bkp)):
                        psq = ps.tile([128, 500], f32, tag="a")
                        for di in range(4):
                            nc.tensor.matmul(
                                psq[:],
                                w[:, di * 256 + p * 128 : di * 256 + (p + 1) * 128],
                                xT[:, di * N + c * 500 : di * N + (c + 1) * 500],
                                start=(di == 0),
                                stop=(di == 3),
                            )
                        nc.vector.tensor_scalar_add(dst[:, cs], psq[:], bias[:, p : p + 1])
                qt_sb.append(qt)
                kt_sb.append(kt)

            # ---- V per head [QB, 24*65]; col 64 of each 65-block stays 1.0 ----
            v_sb = []
            for h in range(HPC):
                v = vpool.tile([QB, NT * 65], bf16, tag=f"v{h}")
                nc.vector.memset(v[:], 1.0)
                v_sb.append(v)
            for p in range(2):
                for t in range(NT):
                    psv = ps.tile([QB, 128], f32, tag="ot")
                    for di in range(4):
                        nc.tensor.matmul(
                            psv[:],
                            xT[:, di * N + t * QB : di * N + (t + 1) * QB],
                            w_sb["v"][:, di * 256 + p * 128 : di * 256 + (p + 1) * 128],
                            start=(di == 0),
                            stop=False,
                        )
                    nc.tensor.matmul(
                        psv[:],
                        ones_sb[:1, :QB],
                        bvr[:1, p * 128 : (p + 1) * 128],
                        start=False,
                        stop=True,
                    )
                    for hh in range(2):
                        h = p * 2 + hh
                        nc.vector.tensor_scalar_max(
                            v_sb[h][:, t * 65 : t * 65 + 64],
                            psv[:, hh * 64 : (hh + 1) * 64],
                            0.0,
                        )

            if debug:
                nc.sync.dma_start(out=dbg["xt"][:], in_=xT[:])
            phase01.close()

            phase2 = ctx.enter_context(contextlib.ExitStack())
            otp = phase2.enter_context(tc.tile_pool(name="otp", bufs=2))
            rcp = phase2.enter_context(tc.tile_pool(name="rcp", bufs=2))
            etp = phase2.enter_context(tc.tile_pool(name="etp", bufs=3))
            attp = phase2.enter_context(tc.tile_pool(name="attp", bufs=2))
            obp = phase2.enter_context(tc.tile_pool(name="obp", bufs=3))
            osg = phase2.enter_context(tc.tile_pool(name="osg", bufs=1))
            ostage = osg.tile([QB, NT * 256], f32, tag="ostage")

            # ---- per head: PV phase then attn phase ----
            for h in range(HPC):
                p, hb = h // 2, (h % 2) * 64
                qt, kt, v = qt_sb[p], kt_sb[p], v_sb[h]

                # PV: S^T[k,q] in q-thirds; exp; accumulate [V|1]^T E^T
                outT = otp.tile([65, N], f32)
                for g in range(3):
                    bps = psb.tile([65, 1024], f32, tag="b")
                    for t in range(NT):
                        aps = ps.tile([QB, 1024], f32, tag="a")
                        for c in range(2):
                            nc.tensor.matmul(
                                aps[:, c * 512 : c * 512 + 500],
                                kt[hb : hb + 64, t * QB : (t + 1) * QB],
                                qt[hb : hb + 64, g * 1000 + c * 500 : g * 1000 + (c + 1) * 500],
                            )
                        if t // 8 == g:
                            off = t * QB - g * 1000
                            ccol = off if off < 500 else off + 12
                            nc.vector.tensor_add(
                                aps[:, ccol : ccol + QB], aps[:, ccol : ccol + QB], mask_sb[:]
                            )
                        et = etp.tile([QB, 1000], bf16)
                        nc.scalar.activation(
                            et[:].rearrange("p (c w) -> p c w", w=500),
                            aps[:].rearrange("p (c w) -> p c w", w=512)[:, :, 0:500],
                            EXP,
                            scale=0.125,
                        )
                        for c in range(2):
                            nc.tensor.matmul(
                                bps[:, c * 512 : c * 512 + 500],
                                v[:, t * 65 : (t + 1) * 65],
                                et[:, c * 500 : (c + 1) * 500],
                                start=(t == 0),
                                stop=(t == NT - 1),
                            )
                    nc.vector.tensor_copy(
                        outT[:, g * 1000 : (g + 1) * 1000].rearrange("p (c w) -> p c w", w=500),
                        bps[:].rearrange("p (c w) -> p c w", w=512)[:, :, 0:500],
                    )

                # transpose out^T blocks; per-tile reciprocal of sums column
                recipT = rcp.tile([QB, NT], f32)
                for t in range(NT):
                    ot = ps.tile([QB, 65], f32, tag="ot")
                    nc.tensor.transpose(
                        ot[:], outT[:, t * QB : (t + 1) * QB], id_sb[:65, :65]
                    )
                    ob = obp.tile([QB, 64], f32)
                    nc.vector.tensor_copy(ob[:], ot[:, 0:64])
                    nc.vector.reciprocal(recipT[:, t : t + 1], ot[:, 64:65])
                    nc.vector.tensor_scalar_mul(
                        ostage[:, t * 256 + h * 64 : t * 256 + (h + 1) * 64],
                        ob[:],
                        recipT[:, t : t + 1],
                    )

                if debug and h == 0:
                    nc.sync.dma_start(out=dbg["qt0"][:], in_=qt_sb[0][:])
                    nc.sync.dma_start(out=dbg["kt0"][:], in_=kt_sb[0][:])
                    nc.sync.dma_start(out=dbg["v0"][:], in_=v_sb[0][:])
                    nc.sync.dma_start(out=dbg["outT0"][:], in_=outT[:])
                    nc.sync.dma_start(out=dbg["recipT0"][:], in_=recipT[:])

                # attn: S[q,k] per q-tile (in k-thirds), exp, normalize, DMA out
                for t in range(NT):
                    att = attp.tile([QB, N], f32)
                    for g in range(3):
                        sa = ps.tile([QB, 1024], f32, tag="a")
                        for c in range(2):
                            nc.tensor.matmul(
                                sa[:, c * 512 : c * 512 + 500],
                                qt[hb : hb + 64, t * QB : (t + 1) * QB],
                                kt[hb : hb + 64, g * 1000 + c * 500 : g * 1000 + (c + 1) * 500],
                            )
                        if t // 8 == g:
                            off = t * QB - g * 1000
                            ccol = off if off < 500 else off + 12
                            nc.vector.tensor_add(
                                sa[:, ccol : ccol + QB], sa[:, ccol : ccol + QB], mask_sb[:]
                            )
                        nc.scalar.activation(
                            att[:, g * 1000 : (g + 1) * 1000].rearrange("p (c w) -> p c w", w=500),
                            sa[:].rearrange("p (c w) -> p c w", w=512)[:, :, 0:500],
                            EXP,
                            scale=0.125,
                        )
                    nc.vector.tensor_scalar_mul(att[:], att[:], recipT[:, t : t + 1])
                    nc.sync.dma_start(
                        out=attn_o[h, t * QB : (t + 1) * QB, :], in_=att[:]
                    )

            # ---- final out DMA ----
            nc.sync.dma_start(
                out=out_o.rearrange("(t p) c -> p t c", p=QB),
                in_=ostage[:].rearrange("p (t c) -> p t c", c=256),
            )

    _fix_multiwait(nc)
    return nc


_CACHE = {}
TRACE = False
TRACE_KWARGS = {}
LAST_RESULT = [None]


def _get_nc():
    if "nc" not in _CACHE:
        _CACHE["nc"] = _build_bass()
    return _CACHE["nc"]


def _mask_np():
    block = np.kron(np.eye(5, dtype=np.float32), np.ones((J, J), np.float32))
    return (NEGM * (block - np.eye(QB, dtype=np.float32))).astype(np.float32)


def kernel(x, Wq, bq, Wk, bk, Wv, bv):
    _install_profile_hook()
    from concourse.bass_utils import run_bass_kernel_spmd
    from concourse import bass_utils

    bass_utils.upload_artifacts = lambda tmpdir: f"local://{tmpdir}"

    x = np.asarray(x, dtype=np.float32)
    Wq, Wk, Wv = (np.asarray(a, np.float32) for a in (Wq, Wk, Wv))
    bq, bk, bv = (np.asarray(a, np.float32) for a in (bq, bk, bv))

    mask = _mask_np()
    ident = np.eye(128, dtype=np.float32)

    in_maps = []
    for c in range(8):
        b, hp = c // 2, c % 2
        cols = slice(hp * 256, (hp + 1) * 256)
        in_maps.append(
            {
                "x": np.ascontiguousarray(x[b]),
                "wq": np.ascontiguousarray(Wq[:, cols]),
                "wk": np.ascontiguousarray(Wk[:, cols]),
                "wv": np.ascontiguousarray(Wv[:, cols]),
                "bq": np.ascontiguousarray(bq[cols]),
                "bk": np.ascontiguousarray(bk[cols]),
                "bv": np.ascontiguousarray(bv[cols]),
                "maskadd": mask,
                "ident": ident,
            }
        )

    nc = _get_nc()
    res = run_bass_kernel_spmd(
        nc, in_maps, list(range(8)), trace=TRACE, **TRACE_KWARGS
    )
    LAST_RESULT[0] = res

    out = np.empty((B, N, 512), np.float32)
    attn = np.empty((B, H, N, N), np.float32)
    for c in range(8):
        b, hp = c // 2, c % 2
        out[b, :, hp * 256 : (hp + 1) * 256] = res.results[c]["out_o"]
        attn[b, hp * 4 : (hp + 1) * 4] = res.results[c]["attn_o"]
    return out, attn


# revision 17
# speedup vs baseline: 1.4415x; 1.0791x over previous
"""Trainium2 Bass kernel for masked multi-head attention (returns out AND attn).

Problem: B=4, N=3000 (120 frames x 25), D=512, H=8, DH=64.
  q/k = x@W+b per head; v = relu(x@Wv+bv)
  scores = q k^T / 8, masked so tokens can't attend within their own frame
  (except self), softmax, out = attn @ v.  Returns (out[B,N,512], attn[B,H,N,N]).

Sharding: 8 cores = (batch b = core//2) x (head-half hp = core%2, 4 heads each).
No cross-core communication.

Per-core plan ("dup-exp"): score matrices are computed on the PE in BOTH
orientations — S[q,k] for the attention output (contiguous HBM rows) and
S^T[k,q] for the P.V matmul (PE contracts over the partition dim, so P.V
needs k on partitions).  exp runs on the scalar engine for both (an exp
costs the same as the copy it replaces).  The in-frame mask is applied by
adding -1e5 to the masked 125x125 block on PSUM before exp (exp underflows
to exactly 0).  Row-sums come free from a ones-column appended to V.
"""

import contextlib
import ctypes
import sys
import types

import numpy as np

B, N, D, H, DH = 4, 3000, 512, 8, 64
J, F = 25, 120
QB = 125          # query/key tile (24 tiles; 125 = 5 frames exactly)
NT = N // QB      # 24
HPC = 4           # heads per core
NEGM = -1.0e5

_SO_PATH = "/opt/axon/libaxon_pjrt.so"


def _install_profile_hook():
    if "antenv.axon_hooks" in sys.modules:
        return
    try:
        lib = ctypes.CDLL(_SO_PATH)
        lib.axon_start_nrt_profile.argtypes = [
            ctypes.POINTER(ctypes.c_int64),
            ctypes.c_size_t,
        ]
        lib.axon_start_nrt_profile.restype = ctypes.c_int64
        lib.axon_stop_nrt_profile.argtypes = [ctypes.c_char_p]
        lib.axon_stop_nrt_profile.restype = ctypes.c_int64
    except OSError:
        return

    @contextlib.contextmanager
    def _hook(output_dir, device_ids):
        import jax

        jax.devices()
        if device_ids:
            ids = (ctypes.c_int64 * len(device_ids))(*device_ids)
            rc = lib.axon_start_nrt_profile(ids, len(device_ids))
        else:
            rc = lib.axon_start_nrt_profile(None, 0)
        if rc != 0:
            raise RuntimeError(f"axon_start_nrt_profile rc={rc}")
        try:
            yield
        finally:
            n = lib.axon_stop_nrt_profile(str(output_dir).encode())
            print(f"profile: {n} file(s) written to {output_dir}")

    mod = types.ModuleType("antenv.axon_hooks")
    mod.get_axon_ntff_profile_hook = lambda: _hook
    mod.set_axon_ntff_profile_hook = lambda h: None
    sys.modules["antenv.axon_hooks"] = mod


def _fix_multiwait(nc):
    """This walrus build accepts one sync wait per instruction; split any
    multi-wait instruction into single-wait EventSemaphore prefixes."""
    from concourse import mybir

    for fn in nc.m.functions:
        for bb in fn.blocks:
            new_list = []
            changed = False
            for inst in bb.instructions:
                si = getattr(inst, "sync_info", None)
                if si is not None and si.on_wait and len(si.on_wait) > 1:
                    waits = list(si.on_wait)
                    for j, w in enumerate(waits[:-1]):
                        new_list.append(
                            mybir.InstEventSemaphore(
                                name=f"{inst.name}-wsplit{j}",
                                engine=inst.engine,
                                ins=[],
                                outs=[],
                                sync_info=mybir.SyncInfo(on_wait=[w], on_update=[]),
                            )
                        )
                    si.on_wait = [waits[-1]]
                    changed = True
                new_list.append(inst)
            if changed:
                bb.instructions[:] = new_list
    return nc


def _build_bass(debug=False):
    import concourse.bass as bass
    import concourse.tile as tile
    from concourse import mybir

    f32 = mybir.dt.float32
    bf16 = mybir.dt.bfloat16
    EXP = mybir.ActivationFunctionType.Exp

    nc = bass.Bass()
    dbg = {}
    if debug:
        dbg["qt0"] = nc.dram_tensor("dbg_qt0", [128, N], f32, kind="ExternalOutput")
        dbg["kt0"] = nc.dram_tensor("dbg_kt0", [128, N], f32, kind="ExternalOutput")
        dbg["v0"] = nc.dram_tensor("dbg_v0", [QB, NT * 64], f32, kind="ExternalOutput")
        dbg["outT0"] = nc.dram_tensor("dbg_outT0", [64, N], f32, kind="ExternalOutput")
        dbg["recipT0"] = nc.dram_tensor("dbg_recipT0", [QB, NT], f32, kind="ExternalOutput")
        dbg["xt"] = nc.dram_tensor("dbg_xt", [128, 4 * N], f32, kind="ExternalOutput")
    x_d = nc.dram_tensor("x", [N, D], f32, kind="ExternalInput")
    wq_d = nc.dram_tensor("wq", [D, 256], f32, kind="ExternalInput")
    wk_d = nc.dram_tensor("wk", [D, 256], f32, kind="ExternalInput")
    wv_d = nc.dram_tensor("wv", [D, 256], f32, kind="ExternalInput")
    bq_d = nc.dram_tensor("bq", [256], f32, kind="ExternalInput")
    bk_d = nc.dram_tensor("bk", [256], f32, kind="ExternalInput")
    bv_d = nc.dram_tensor("bv", [256], f32, kind="ExternalInput")
    mask_d = nc.dram_tensor("maskadd", [QB, QB], f32, kind="ExternalInput")
    id_d = nc.dram_tensor("ident", [128, 128], f32, kind="ExternalInput")
    attn_o = nc.dram_tensor("attn_o", [HPC, N, N], f32, kind="ExternalOutput")
    out_o = nc.dram_tensor("out_o", [N, 256], f32, kind="ExternalOutput")

    with tile.TileContext(nc) as tc:
        with contextlib.ExitStack() as ctx:
            cst = ctx.enter_context(tc.tile_pool(name="cst", bufs=1))
            qkt = ctx.enter_context(tc.tile_pool(name="qkt", bufs=1))
            vpool = ctx.enter_context(tc.tile_pool(name="vpool", bufs=1))
            # Shared PSUM pool: tag "a" 2 banks x2, "b" 2 banks x1,
            # "ot" 1 bank x2 -> 8 banks exactly.
            ps = ctx.enter_context(tc.tile_pool(name="ps", bufs=2, space="PSUM"))
            psb = ctx.enter_context(tc.tile_pool(name="psb", bufs=1, space="PSUM"))

            phase01 = ctx.enter_context(contextlib.ExitStack())
            wpool = phase01.enter_context(tc.tile_pool(name="wpool", bufs=1))
            xtp = phase01.enter_context(tc.tile_pool(name="xtp", bufs=1))
            xin = phase01.enter_context(tc.tile_pool(name="xin", bufs=3))

            # ---- constants ----
            mask_sb = cst.tile([QB, QB], f32, tag="mask")
            nc.sync.dma_start(out=mask_sb[:], in_=mask_d[:])
            id_sb = cst.tile([128, 128], f32, tag="ident")
            nc.sync.dma_start(out=id_sb[:], in_=id_d[:])
            ones_sb = cst.tile([1, QB], bf16, tag="ones")
            nc.vector.memset(ones_sb[:], 1.0)
            bqp = cst.tile([128, 2], f32, tag="bqp")
            bkp = cst.tile([128, 2], f32, tag="bkp")
            nc.sync.dma_start(out=bqp[:], in_=bq_d.rearrange("(p d) -> d p", p=2))
            nc.sync.dma_start(out=bkp[:], in_=bk_d.rearrange("(p d) -> d p", p=2))
            bvr = cst.tile([1, 256], bf16, tag="bvr")
            nc.gpsimd.dma_start(out=bvr[:], in_=bv_d.rearrange("(o c) -> o c", o=1))

            # ---- weights ----
            w_sb = {}
            for nm, wd in (("q", wq_d), ("k", wk_d), ("v", wv_d)):
                w = wpool.tile([128, 4 * 256], bf16, tag=f"w{nm}")
                for di in range(4):
                    nc.gpsimd.dma_start(
                        out=w[:, di * 256 : (di + 1) * 256],
                        in_=wd[di * 128 : (di + 1) * 128, :],
                    )
                w_sb[nm] = w

            # ---- x -> xT  (xT[:, di*N + t*QB + j] = x[t*QB + j, di*128 + p]) ----
            xT = xtp.tile([128, 4 * N], bf16, tag="xT")
            for t in range(NT):
                xt_in = xin.tile([QB, D], f32)
                nc.sync.dma_start(out=xt_in[:], in_=x_d[t * QB : (t + 1) * QB, :])
                for di in range(4):
                    pt = ps.tile([128, QB], f32, tag="ot")
                    nc.tensor.transpose(
                        pt[:], xt_in[:, di * 128 : (di + 1) * 128], id_sb[:QB, :QB]
                    )
                    nc.vector.tensor_copy(
                        xT[:, di * N + t * QB : di * N + (t + 1) * QB], pt[:]
                    )

            # ---- projections: QT/KT per pair [128, N] (two heads stacked) ----
            qt_sb = []
            kt_sb = []
            for p in range(2):
                qt = qkt.tile([128, N], bf16, tag=f"qt{p}")
                kt = qkt.tile([128, N], bf16, tag=f"kt{p}")
                for c in range(6):
                    cs = slice(c * 500, (c + 1) * 500)
                    for dst, w, bias in ((qt, w_sb["q"], bqp), (kt, w_sb["k"], bkp)):
                        psq = ps.tile([128, 500], f32, tag="a")
                        for di in range(4):
                            nc.tensor.matmul(
                                psq[:],
                                w[:, di * 256 + p * 128 : di * 256 + (p + 1) * 128],
                                xT[:, di * N + c * 500 : di * N + (c + 1) * 500],
                                start=(di == 0),
                                stop=(di == 3),
                            )
                        nc.vector.tensor_scalar_add(dst[:, cs], psq[:], bias[:, p : p + 1])
                qt_sb.append(qt)
                kt_sb.append(kt)

            # ---- V per head [QB, 24*65]; col 64 of each 65-block stays 1.0 ----
            v_sb = []
            for h in range(HPC):
                v = vpool.tile([QB, NT * 64], bf16, tag=f"v{h}")
                v_sb.append(v)
            for p in range(2):
                for t in range(NT):
                    psv = ps.tile([QB, 128], f32, tag="ot")
                    for di in range(4):
                        nc.tensor.matmul(
                            psv[:],
                            xT[:, di * N + t * QB : di * N + (t + 1) * QB],
                            w_sb["v"][:, di * 256 + p * 128 : di * 256 + (p + 1) * 128],
                            start=(di == 0),
                            stop=False,
                        )
                    nc.tensor.matmul(
                        psv[:],
                        ones_sb[:1, :QB],
                        bvr[:1, p * 128 : (p + 1) * 128],
                        start=False,
                        stop=True,
                    )
                    for hh in range(2):
                        h = p * 2 + hh
                        nc.vector.tensor_scalar_max(
                            v_sb[h][:, t * 64 : (t + 1) * 64],
                            psv[:, hh * 64 : (hh + 1) * 64],
                            0.0,
                        )

            if debug:
                nc.sync.dma_start(out=dbg["xt"][:], in_=xT[:])
            phase01.close()

            phase2 = ctx.enter_context(contextlib.ExitStack())
            otp = phase2.enter_context(tc.tile_pool(name="otp", bufs=2))
            rcp = phase2.enter_context(tc.tile_pool(name="rcp", bufs=2))
            etp = phase2.enter_context(tc.tile_pool(name="etp", bufs=4))
            attp = phase2.enter_context(tc.tile_pool(name="attp", bufs=4))
            obp = phase2.enter_context(tc.tile_pool(name="obp", bufs=3))
            accp = phase2.enter_context(tc.tile_pool(name="accp", bufs=4))
            osg = phase2.enter_context(tc.tile_pool(name="osg", bufs=1))
            ostage = osg.tile([QB, NT * 256], f32, tag="ostage")

            # ---- per head: PV phase then attn phase ----
            for h in range(HPC):
                p, hb = h // 2, (h % 2) * 64
                qt, kt, v = qt_sb[p], kt_sb[p], v_sb[h]

                # PV: S^T[k,q] in q-thirds; exp; accumulate [V|1]^T E^T
                outT = otp.tile([64, N], f32)
                for g in range(3):
                    bps = psb.tile([64, 1024], f32, tag="b")
                    for t in range(NT):
                        aps = ps.tile([QB, 1024], f32, tag="a")
                        for c in range(2):
                            nc.tensor.matmul(
                                aps[:, c * 512 : c * 512 + 500],
                                kt[hb : hb + 64, t * QB : (t + 1) * QB],
                                qt[hb : hb + 64, g * 1000 + c * 500 : g * 1000 + (c + 1) * 500],
                            )
                        if t // 8 == g:
                            off = t * QB - g * 1000
                            ccol = off if off < 500 else off + 12
                            nc.vector.tensor_add(
                                aps[:, ccol : ccol + QB], aps[:, ccol : ccol + QB], mask_sb[:]
                            )
                        et = etp.tile([QB, 1000], bf16)
                        nc.scalar.activation(
                            et[:].rearrange("p (c w) -> p c w", w=500),
                            aps[:].rearrange("p (c w) -> p c w", w=512)[:, :, 0:500],
                            EXP,
                            scale=0.125,
                        )
                        for c in range(2):
                            nc.tensor.matmul(
                                bps[:, c * 512 : c * 512 + 500],
                                v[:, t * 64 : (t + 1) * 64],
                                et[:, c * 500 : (c + 1) * 500],
                                start=(t == 0),
                                stop=(t == NT - 1),
                            )
                    nc.vector.tensor_copy(
                        outT[:, g * 1000 : (g + 1) * 1000].rearrange("p (c w) -> p c w", w=500),
                        bps[:].rearrange("p (c w) -> p c w", w=512)[:, :, 0:500],
                    )

                # attn: S[q,k] per q-tile (in k-thirds), exp (+row-sum accum),
                # normalize, DMA out.  Row-sums come from accum_out, so this
                # phase is independent of the PV phase.
                recipT = rcp.tile([QB, NT], f32)
                for t in range(NT):
                    att = attp.tile([QB, N], f32)
                    acc = accp.tile([QB, 3], f32)
                    for g in range(3):
                        sa = ps.tile([QB, 1024], f32, tag="a")
                        for c in range(2):
                            nc.tensor.matmul(
                                sa[:, c * 512 : c * 512 + 500],
                                qt[hb : hb + 64, t * QB : (t + 1) * QB],
                                kt[hb : hb + 64, g * 1000 + c * 500 : g * 1000 + (c + 1) * 500],
                            )
                        if t // 8 == g:
                            off = t * QB - g * 1000
                            ccol = off if off < 500 else off + 12
                            nc.vector.tensor_add(
                                sa[:, ccol : ccol + QB], sa[:, ccol : ccol + QB], mask_sb[:]
                            )
                        nc.scalar.activation(
                            att[:, g * 1000 : (g + 1) * 1000].rearrange("p (c w) -> p c w", w=500),
                            sa[:].rearrange("p (c w) -> p c w", w=512)[:, :, 0:500],
                            EXP,
                            scale=0.125,
                            accum_out=acc[:, g : g + 1],
                        )
                    nc.vector.tensor_add(acc[:, 0:1], acc[:, 0:1], acc[:, 1:2])
                    nc.vector.tensor_add(acc[:, 0:1], acc[:, 0:1], acc[:, 2:3])
                    nc.vector.reciprocal(recipT[:, t : t + 1], acc[:, 0:1])
                    nc.vector.tensor_scalar_mul(att[:], att[:], recipT[:, t : t + 1])
                    nc.sync.dma_start(
                        out=attn_o[h, t * QB : (t + 1) * QB, :], in_=att[:]
                    )

                # out epilogue: transpose out^T blocks, scale rows by recip
                for t in range(NT):
                    ot = ps.tile([QB, 64], f32, tag="ot")
                    nc.tensor.transpose(
                        ot[:], outT[:, t * QB : (t + 1) * QB], id_sb[:64, :64]
                    )
                    nc.vector.tensor_scalar_mul(
                        ostage[:, t * 256 + h * 64 : t * 256 + (h + 1) * 64],
                        ot[:],
                        recipT[:, t : t + 1],
                    )

                if debug and h == 0:
                    nc.sync.dma_start(out=dbg["qt0"][:], in_=qt_sb[0][:])
                    nc.sync.dma_start(out=dbg["kt0"][:], in_=kt_sb[0][:])
                    nc.sync.dma_start(out=dbg["v0"][:], in_=v_sb[0][:])
                    nc.sync.dma_start(out=dbg["outT0"][:], in_=outT[:])
                    nc.sync.dma_start(out=dbg["recipT0"][:], in_=recipT[:])

            # ---- final out DMA ----
            nc.sync.dma_start(
                out=out_o.rearrange("(t p) c -> p t c", p=QB),
                in_=ostage[:].rearrange("p (t c) -> p t c", c=256),
            )

    _fix_multiwait(nc)
    return nc


_CACHE = {}
TRACE = False
TRACE_KWARGS = {}
LAST_RESULT = [None]


def _get_nc():
    if "nc" not in _CACHE:
        _CACHE["nc"] = _build_bass()
    return _CACHE["nc"]


def _mask_np():
    block = np.kron(np.eye(5, dtype=np.float32), np.ones((J, J), np.float32))
    return (NEGM * (block - np.eye(QB, dtype=np.float32))).astype(np.float32)


def kernel(x, Wq, bq, Wk, bk, Wv, bv):
    _install_profile_hook()
    from concourse.bass_utils import run_bass_kernel_spmd
    from concourse import bass_utils

    bass_utils.upload_artifacts = lambda tmpdir: f"local://{tmpdir}"

    x = np.asarray(x, dtype=np.float32)
    Wq, Wk, Wv = (np.asarray(a, np.float32) for a in (Wq, Wk, Wv))
    bq, bk, bv = (np.asarray(a, np.float32) for a in (bq, bk, bv))

    mask = _mask_np()
    ident = np.eye(128, dtype=np.float32)

    in_maps = []
    for c in range(8):
        b, hp = c // 2, c % 2
        cols = slice(hp * 256, (hp + 1) * 256)
        in_maps.append(
            {
                "x": np.ascontiguousarray(x[b]),
                "wq": np.ascontiguousarray(Wq[:, cols]),
                "wk": np.ascontiguousarray(Wk[:, cols]),
                "wv": np.ascontiguousarray(Wv[:, cols]),
                "bq": np.ascontiguousarray(bq[cols]),
                "bk": np.ascontiguousarray(bk[cols]),
                "bv": np.ascontiguousarray(bv[cols]),
                "maskadd": mask,
                "ident": ident,
            }
        )

    nc = _get_nc()
    res = run_bass_kernel_spmd(
        nc, in_maps, list(range(8)), trace=TRACE, **TRACE_KWARGS
    )
    LAST_RESULT[0] = res

    out = np.empty((B, N, 512), np.float32)
    attn = np.empty((B, H, N, N), np.float32)
    for c in range(8):
        b, hp = c // 2, c % 2
        out[b, :, hp * 256 : (hp + 1) * 256] = res.results[c]["out_o"]
        attn[b, hp * 4 : (hp + 1) * 4] = res.results[c]["attn_o"]
    return out, attn


# revision 18
# speedup vs baseline: 1.8582x; 1.2891x over previous
"""Trainium2 Bass kernel for masked multi-head attention (returns out AND attn).

Problem: B=4, N=3000 (120 frames x 25), D=512, H=8, DH=64.
  q/k = x@W+b per head; v = relu(x@Wv+bv)
  scores = q k^T / 8, masked so tokens can't attend within their own frame
  (except self), softmax, out = attn @ v.  Returns (out[B,N,512], attn[B,H,N,N]).

Sharding: 8 cores = (batch b = core//2) x (head-half hp = core%2, 4 heads each).
No cross-core communication.

Per-core plan ("dup-exp"): score matrices are computed on the PE in BOTH
orientations — S[q,k] for the attention output (contiguous HBM rows) and
S^T[k,q] for the P.V matmul (PE contracts over the partition dim, so P.V
needs k on partitions).  exp runs on the scalar engine for both (an exp
costs the same as the copy it replaces).  The in-frame mask is applied by
adding -1e5 to the masked 125x125 block on PSUM before exp (exp underflows
to exactly 0).  Row-sums come free from a ones-column appended to V.
"""

import contextlib
import ctypes
import sys
import types

import numpy as np

B, N, D, H, DH = 4, 3000, 512, 8, 64
J, F = 25, 120
QB = 125          # query/key tile (24 tiles; 125 = 5 frames exactly)
NT = N // QB      # 24
HPC = 4           # heads per core
NEGM = -1.0e5

_SO_PATH = "/opt/axon/libaxon_pjrt.so"


def _install_profile_hook():
    if "antenv.axon_hooks" in sys.modules:
        return
    try:
        lib = ctypes.CDLL(_SO_PATH)
        lib.axon_start_nrt_profile.argtypes = [
            ctypes.POINTER(ctypes.c_int64),
            ctypes.c_size_t,
        ]
        lib.axon_start_nrt_profile.restype = ctypes.c_int64
        lib.axon_stop_nrt_profile.argtypes = [ctypes.c_char_p]
        lib.axon_stop_nrt_profile.restype = ctypes.c_int64
    except OSError:
        return

    @contextlib.contextmanager
    def _hook(output_dir, device_ids):
        import jax

        jax.devices()
        if device_ids:
            ids = (ctypes.c_int64 * len(device_ids))(*device_ids)
            rc = lib.axon_start_nrt_profile(ids, len(device_ids))
        else:
            rc = lib.axon_start_nrt_profile(None, 0)
        if rc != 0:
            raise RuntimeError(f"axon_start_nrt_profile rc={rc}")
        try:
            yield
        finally:
            n = lib.axon_stop_nrt_profile(str(output_dir).encode())
            print(f"profile: {n} file(s) written to {output_dir}")

    mod = types.ModuleType("antenv.axon_hooks")
    mod.get_axon_ntff_profile_hook = lambda: _hook
    mod.set_axon_ntff_profile_hook = lambda h: None
    sys.modules["antenv.axon_hooks"] = mod


def _fix_multiwait(nc):
    """This walrus build accepts one sync wait per instruction; split any
    multi-wait instruction into single-wait EventSemaphore prefixes."""
    from concourse import mybir

    for fn in nc.m.functions:
        for bb in fn.blocks:
            new_list = []
            changed = False
            for inst in bb.instructions:
                si = getattr(inst, "sync_info", None)
                if si is not None and si.on_wait and len(si.on_wait) > 1:
                    waits = list(si.on_wait)
                    for j, w in enumerate(waits[:-1]):
                        new_list.append(
                            mybir.InstEventSemaphore(
                                name=f"{inst.name}-wsplit{j}",
                                engine=inst.engine,
                                ins=[],
                                outs=[],
                                sync_info=mybir.SyncInfo(on_wait=[w], on_update=[]),
                            )
                        )
                    si.on_wait = [waits[-1]]
                    changed = True
                new_list.append(inst)
            if changed:
                bb.instructions[:] = new_list
    return nc


def _build_bass(debug=False):
    import concourse.bass as bass
    import concourse.tile as tile
    from concourse import mybir

    f32 = mybir.dt.float32
    bf16 = mybir.dt.bfloat16
    EXP = mybir.ActivationFunctionType.Exp

    nc = bass.Bass()
    dbg = {}
    if debug:
        dbg["qt0"] = nc.dram_tensor("dbg_qt0", [128, N], f32, kind="ExternalOutput")
        dbg["kt0"] = nc.dram_tensor("dbg_kt0", [128, N], f32, kind="ExternalOutput")
        dbg["v0"] = nc.dram_tensor("dbg_v0", [QB, NT * 64], f32, kind="ExternalOutput")
        dbg["outT0"] = nc.dram_tensor("dbg_outT0", [64, N], f32, kind="ExternalOutput")
        dbg["recipT0"] = nc.dram_tensor("dbg_recipT0", [QB, NT], f32, kind="ExternalOutput")
        dbg["xt"] = nc.dram_tensor("dbg_xt", [128, 4 * N], f32, kind="ExternalOutput")
    x_d = nc.dram_tensor("x", [N, D], f32, kind="ExternalInput")
    wq_d = nc.dram_tensor("wq", [D, 256], f32, kind="ExternalInput")
    wk_d = nc.dram_tensor("wk", [D, 256], f32, kind="ExternalInput")
    wv_d = nc.dram_tensor("wv", [D, 256], f32, kind="ExternalInput")
    bq_d = nc.dram_tensor("bq", [256], f32, kind="ExternalInput")
    bk_d = nc.dram_tensor("bk", [256], f32, kind="ExternalInput")
    bv_d = nc.dram_tensor("bv", [256], f32, kind="ExternalInput")
    mask_d = nc.dram_tensor("maskadd", [QB, QB], f32, kind="ExternalInput")
    id_d = nc.dram_tensor("ident", [128, 128], f32, kind="ExternalInput")
    attn_o = nc.dram_tensor("attn_o", [HPC, N, N], f32, kind="ExternalOutput")
    out_o = nc.dram_tensor("out_o", [N, 256], f32, kind="ExternalOutput")

    with tile.TileContext(nc) as tc:
        with contextlib.ExitStack() as ctx:
            cst = ctx.enter_context(tc.tile_pool(name="cst", bufs=1))
            qkt = ctx.enter_context(tc.tile_pool(name="qkt", bufs=1))
            vpool = ctx.enter_context(tc.tile_pool(name="vpool", bufs=1))
            # Shared PSUM pool: tag "a" 2 banks x2, "b" 2 banks x1,
            # "ot" 1 bank x2 -> 8 banks exactly.
            ps = ctx.enter_context(tc.tile_pool(name="ps", bufs=2, space="PSUM"))
            psb = ctx.enter_context(tc.tile_pool(name="psb", bufs=1, space="PSUM"))

            phase01 = ctx.enter_context(contextlib.ExitStack())
            wpool = phase01.enter_context(tc.tile_pool(name="wpool", bufs=1))
            xtp = phase01.enter_context(tc.tile_pool(name="xtp", bufs=1))
            xin = phase01.enter_context(tc.tile_pool(name="xin", bufs=3))

            # ---- constants ----
            mask_sb = cst.tile([QB, QB], f32, tag="mask")
            nc.sync.dma_start(out=mask_sb[:], in_=mask_d[:])
            id_sb = cst.tile([128, 128], f32, tag="ident")
            nc.sync.dma_start(out=id_sb[:], in_=id_d[:])
            ones_sb = cst.tile([1, QB], bf16, tag="ones")
            nc.vector.memset(ones_sb[:], 1.0)
            bqp = cst.tile([128, 2], f32, tag="bqp")
            bkp = cst.tile([128, 2], f32, tag="bkp")
            nc.sync.dma_start(out=bqp[:], in_=bq_d.rearrange("(p d) -> d p", p=2))
            nc.sync.dma_start(out=bkp[:], in_=bk_d.rearrange("(p d) -> d p", p=2))
            bvr = cst.tile([1, 256], bf16, tag="bvr")
            nc.gpsimd.dma_start(out=bvr[:], in_=bv_d.rearrange("(o c) -> o c", o=1))

            # ---- weights ----
            w_sb = {}
            for nm, wd in (("q", wq_d), ("k", wk_d), ("v", wv_d)):
                w = wpool.tile([128, 4 * 256], bf16, tag=f"w{nm}")
                for di in range(4):
                    nc.gpsimd.dma_start(
                        out=w[:, di * 256 : (di + 1) * 256],
                        in_=wd[di * 128 : (di + 1) * 128, :],
                    )
                w_sb[nm] = w

            # ---- x -> xT  (xT[:, di*N + t*QB + j] = x[t*QB + j, di*128 + p]) ----
            xT = xtp.tile([128, 4 * N], bf16, tag="xT")
            for t in range(NT):
                xt_in = xin.tile([QB, D], f32)
                nc.sync.dma_start(out=xt_in[:], in_=x_d[t * QB : (t + 1) * QB, :])
                for di in range(4):
                    pt = ps.tile([128, QB], f32, tag="ot")
                    nc.tensor.transpose(
                        pt[:], xt_in[:, di * 128 : (di + 1) * 128], id_sb[:QB, :QB]
                    )
                    nc.vector.tensor_copy(
                        xT[:, di * N + t * QB : di * N + (t + 1) * QB], pt[:]
                    )

            # ---- projections: QT/KT per pair [128, N] (two heads stacked) ----
            qt_sb = []
            kt_sb = []
            for p in range(2):
                qt = qkt.tile([128, N], bf16, tag=f"qt{p}")
                kt = qkt.tile([128, N], bf16, tag=f"kt{p}")
                for c in range(6):
                    cs = slice(c * 500, (c + 1) * 500)
                    for dst, w, bias in ((qt, w_sb["q"], bqp), (kt, w_sb["k"], bkp)):
                        psq = ps.tile([128, 500], f32, tag="a")
                        for di in range(4):
                            nc.tensor.matmul(
                                psq[:],
                                w[:, di * 256 + p * 128 : di * 256 + (p + 1) * 128],
                                xT[:, di * N + c * 500 : di * N + (c + 1) * 500],
                                start=(di == 0),
                                stop=(di == 3),
                            )
                        nc.vector.tensor_scalar_add(dst[:, cs], psq[:], bias[:, p : p + 1])
                qt_sb.append(qt)
                kt_sb.append(kt)

            # ---- V per head [QB, 24*65]; col 64 of each 65-block stays 1.0 ----
            v_sb = []
            for h in range(HPC):
                v = vpool.tile([QB, NT * 64], bf16, tag=f"v{h}")
                v_sb.append(v)
            for p in range(2):
                for t in range(NT):
                    psv = ps.tile([QB, 128], f32, tag="ot")
                    for di in range(4):
                        nc.tensor.matmul(
                            psv[:],
                            xT[:, di * N + t * QB : di * N + (t + 1) * QB],
                            w_sb["v"][:, di * 256 + p * 128 : di * 256 + (p + 1) * 128],
                            start=(di == 0),
                            stop=False,
                        )
                    nc.tensor.matmul(
                        psv[:],
                        ones_sb[:1, :QB],
                        bvr[:1, p * 128 : (p + 1) * 128],
                        start=False,
                        stop=True,
                    )
                    for hh in range(2):
                        h = p * 2 + hh
                        nc.vector.tensor_scalar_max(
                            v_sb[h][:, t * 64 : (t + 1) * 64],
                            psv[:, hh * 64 : (hh + 1) * 64],
                            0.0,
                        )

            if debug:
                nc.sync.dma_start(out=dbg["xt"][:], in_=xT[:])
            phase01.close()

            phase2 = ctx.enter_context(contextlib.ExitStack())
            otp = phase2.enter_context(tc.tile_pool(name="otp", bufs=2))
            rcp = phase2.enter_context(tc.tile_pool(name="rcp", bufs=2))
            etp = phase2.enter_context(tc.tile_pool(name="etp", bufs=4))
            attp = phase2.enter_context(tc.tile_pool(name="attp", bufs=4))
            obp = phase2.enter_context(tc.tile_pool(name="obp", bufs=3))
            accp = phase2.enter_context(tc.tile_pool(name="accp", bufs=4))
            osg = phase2.enter_context(tc.tile_pool(name="osg", bufs=1))
            ostage = osg.tile([QB, NT * 256], f32, tag="ostage")

            # ---- per head: PV phase then attn phase ----
            for h in range(HPC):
                p, hb = h // 2, (h % 2) * 64
                qt, kt, v = qt_sb[p], kt_sb[p], v_sb[h]

                # PV: S^T[k,q] in q-thirds; exp; accumulate [V|1]^T E^T
                outT = otp.tile([64, N], f32)
                for g in range(3):
                    bps = psb.tile([64, 1024], f32, tag="b")
                    for t in range(NT):
                        aps = ps.tile([QB, 1024], f32, tag="a")
                        for c in range(2):
                            nc.tensor.matmul(
                                aps[:, c * 512 : c * 512 + 500],
                                kt[hb : hb + 64, t * QB : (t + 1) * QB],
                                qt[hb : hb + 64, g * 1000 + c * 500 : g * 1000 + (c + 1) * 500],
                            )
                        if t // 8 == g:
                            off = t * QB - g * 1000
                            ccol = off if off < 500 else off + 12
                            nc.vector.tensor_add(
                                aps[:, ccol : ccol + QB], aps[:, ccol : ccol + QB], mask_sb[:]
                            )
                        et = etp.tile([QB, 1000], bf16)
                        nc.scalar.activation(
                            et[:].rearrange("p (c w) -> p c w", w=500),
                            aps[:].rearrange("p (c w) -> p c w", w=512)[:, :, 0:500],
                            EXP,
                            scale=0.125,
                        )
                        for c in range(2):
                            nc.tensor.matmul(
                                bps[:, c * 512 : c * 512 + 500],
                                v[:, t * 64 : (t + 1) * 64],
                                et[:, c * 500 : (c + 1) * 500],
                                start=(t == 0),
                                stop=(t == NT - 1),
                            )
                    nc.vector.tensor_copy(
                        outT[:, g * 1000 : (g + 1) * 1000].rearrange("p (c w) -> p c w", w=500),
                        bps[:].rearrange("p (c w) -> p c w", w=512)[:, :, 0:500],
                    )

                # attn: S[q,k] per q-tile (in k-thirds), exp (+row-sum accum),
                # normalize, DMA out.  Row-sums come from accum_out, so this
                # phase is independent of the PV phase.
                recipT = rcp.tile([QB, NT], f32)
                for t in range(NT):
                    att = attp.tile([QB, N], f32)
                    acc = accp.tile([QB, 3], f32)
                    for g in range(3):
                        sa = ps.tile([QB, 1024], f32, tag="a")
                        for c in range(2):
                            nc.tensor.matmul(
                                sa[:, c * 512 : c * 512 + 500],
                                qt[hb : hb + 64, t * QB : (t + 1) * QB],
                                kt[hb : hb + 64, g * 1000 + c * 500 : g * 1000 + (c + 1) * 500],
                            )
                        if t // 8 == g:
                            off = t * QB - g * 1000
                            ccol = off if off < 500 else off + 12
                            nc.vector.tensor_add(
                                sa[:, ccol : ccol + QB], sa[:, ccol : ccol + QB], mask_sb[:]
                            )
                        nc.scalar.activation(
                            att[:, g * 1000 : (g + 1) * 1000].rearrange("p (c w) -> p c w", w=500),
                            sa[:].rearrange("p (c w) -> p c w", w=512)[:, :, 0:500],
                            EXP,
                            scale=0.125,
                            accum_out=acc[:, g : g + 1],
                        )
                    nc.vector.tensor_add(acc[:, 0:1], acc[:, 0:1], acc[:, 1:2])
                    nc.vector.tensor_add(acc[:, 0:1], acc[:, 0:1], acc[:, 2:3])
                    nc.vector.reciprocal(recipT[:, t : t + 1], acc[:, 0:1])
                    nc.vector.tensor_scalar_mul(att[:], att[:], recipT[:, t : t + 1])
                    nc.gpsimd.dma_start(
                        out=attn_o[h, t * QB : (t + 1) * QB, :], in_=att[:]
                    )

                # out epilogue: transpose out^T blocks, scale rows by recip
                for t in range(NT):
                    ot = ps.tile([QB, 64], f32, tag="ot")
                    nc.tensor.transpose(
                        ot[:], outT[:, t * QB : (t + 1) * QB], id_sb[:64, :64]
                    )
                    nc.vector.tensor_scalar_mul(
                        ostage[:, t * 256 + h * 64 : t * 256 + (h + 1) * 64],
                        ot[:],
                        recipT[:, t : t + 1],
                    )

                if debug and h == 0:
                    nc.sync.dma_start(out=dbg["qt0"][:], in_=qt_sb[0][:])
                    nc.sync.dma_start(out=dbg["kt0"][:], in_=kt_sb[0][:])
                    nc.sync.dma_start(out=dbg["v0"][:], in_=v_sb[0][:])
                    nc.sync.dma_start(out=dbg["outT0"][:], in_=outT[:])
                    nc.sync.dma_start(out=dbg["recipT0"][:], in_=recipT[:])

            # ---- final out DMA ----
            nc.gpsimd.dma_start(
                out=out_o.rearrange("(t p) c -> p t c", p=QB),
                in_=ostage[:].rearrange("p (t c) -> p t c", c=256),
            )

    _fix_multiwait(nc)
    return nc


_CACHE = {}
TRACE = False
TRACE_KWARGS = {}
LAST_RESULT = [None]


def _get_nc():
    if "nc" not in _CACHE:
        _CACHE["nc"] = _build_bass()
    return _CACHE["nc"]


def _mask_np():
    block = np.kron(np.eye(5, dtype=np.float32), np.ones((J, J), np.float32))
    return (NEGM * (block - np.eye(QB, dtype=np.float32))).astype(np.float32)


def kernel(x, Wq, bq, Wk, bk, Wv, bv):
    _install_profile_hook()
    from concourse.bass_utils import run_bass_kernel_spmd
    from concourse import bass_utils

    bass_utils.upload_artifacts = lambda tmpdir: f"local://{tmpdir}"

    x = np.asarray(x, dtype=np.float32)
    Wq, Wk, Wv = (np.asarray(a, np.float32) for a in (Wq, Wk, Wv))
    bq, bk, bv = (np.asarray(a, np.float32) for a in (bq, bk, bv))

    mask = _mask_np()
    ident = np.eye(128, dtype=np.float32)

    in_maps = []
    for c in range(8):
        b, hp = c // 2, c % 2
        cols = slice(hp * 256, (hp + 1) * 256)
        in_maps.append(
            {
                "x": np.ascontiguousarray(x[b]),
                "wq": np.ascontiguousarray(Wq[:, cols]),
                "wk": np.ascontiguousarray(Wk[:, cols]),
                "wv": np.ascontiguousarray(Wv[:, cols]),
                "bq": np.ascontiguousarray(bq[cols]),
                "bk": np.ascontiguousarray(bk[cols]),
                "bv": np.ascontiguousarray(bv[cols]),
                "maskadd": mask,
                "ident": ident,
            }
        )

    nc = _get_nc()
    res = run_bass_kernel_spmd(
        nc, in_maps, list(range(8)), trace=TRACE, **TRACE_KWARGS
    )
    LAST_RESULT[0] = res

    out = np.empty((B, N, 512), np.float32)
    attn = np.empty((B, H, N, N), np.float32)
    for c in range(8):
        b, hp = c // 2, c % 2
        out[b, :, hp * 256 : (hp + 1) * 256] = res.results[c]["out_o"]
        attn[b, hp * 4 : (hp + 1) * 4] = res.results[c]["attn_o"]
    return out, attn


# revision 19
# speedup vs baseline: 2.1978x; 1.1828x over previous
"""Trainium2 Bass kernel for masked multi-head attention (returns out AND attn).

Problem: B=4, N=3000 (120 frames x 25), D=512, H=8, DH=64.
  q/k = x@W+b per head; v = relu(x@Wv+bv)
  scores = q k^T / 8, masked so tokens can't attend within their own frame
  (except self), softmax, out = attn @ v.  Returns (out[B,N,512], attn[B,H,N,N]).

Sharding: 8 cores = (batch b = core//2) x (head-half hp = core%2, 4 heads each).
No cross-core communication.

Per-core plan ("dup-exp"): score matrices are computed on the PE in BOTH
orientations — S[q,k] for the attention output (contiguous HBM rows) and
S^T[k,q] for the P.V matmul (PE contracts over the partition dim, so P.V
needs k on partitions).  exp runs on the scalar engine for both (an exp
costs the same as the copy it replaces).  The in-frame mask is applied by
adding -1e5 to the masked 125x125 block on PSUM before exp (exp underflows
to exactly 0).  Row-sums come free from a ones-column appended to V.
"""

import contextlib
import ctypes
import sys
import types

import numpy as np

B, N, D, H, DH = 4, 3000, 512, 8, 64
J, F = 25, 120
QB = 125          # query/key tile (24 tiles; 125 = 5 frames exactly)
NT = N // QB      # 24
HPC = 4           # heads per core
NEGM = -1.0e5

_SO_PATH = "/opt/axon/libaxon_pjrt.so"


def _install_profile_hook():
    if "antenv.axon_hooks" in sys.modules:
        return
    try:
        lib = ctypes.CDLL(_SO_PATH)
        lib.axon_start_nrt_profile.argtypes = [
            ctypes.POINTER(ctypes.c_int64),
            ctypes.c_size_t,
        ]
        lib.axon_start_nrt_profile.restype = ctypes.c_int64
        lib.axon_stop_nrt_profile.argtypes = [ctypes.c_char_p]
        lib.axon_stop_nrt_profile.restype = ctypes.c_int64
    except OSError:
        return

    @contextlib.contextmanager
    def _hook(output_dir, device_ids):
        import jax

        jax.devices()
        if device_ids:
            ids = (ctypes.c_int64 * len(device_ids))(*device_ids)
            rc = lib.axon_start_nrt_profile(ids, len(device_ids))
        else:
            rc = lib.axon_start_nrt_profile(None, 0)
        if rc != 0:
            raise RuntimeError(f"axon_start_nrt_profile rc={rc}")
        try:
            yield
        finally:
            n = lib.axon_stop_nrt_profile(str(output_dir).encode())
            print(f"profile: {n} file(s) written to {output_dir}")

    mod = types.ModuleType("antenv.axon_hooks")
    mod.get_axon_ntff_profile_hook = lambda: _hook
    mod.set_axon_ntff_profile_hook = lambda h: None
    sys.modules["antenv.axon_hooks"] = mod


def _fix_multiwait(nc):
    """This walrus build accepts one sync wait per instruction; split any
    multi-wait instruction into single-wait EventSemaphore prefixes."""
    from concourse import mybir

    for fn in nc.m.functions:
        for bb in fn.blocks:
            new_list = []
            changed = False
            for inst in bb.instructions:
                si = getattr(inst, "sync_info", None)
                if si is not None and si.on_wait and len(si.on_wait) > 1:
                    waits = list(si.on_wait)
                    for j, w in enumerate(waits[:-1]):
                        new_list.append(
                            mybir.InstEventSemaphore(
                                name=f"{inst.name}-wsplit{j}",
                                engine=inst.engine,
                                ins=[],
                                outs=[],
                                sync_info=mybir.SyncInfo(on_wait=[w], on_update=[]),
                            )
                        )
                    si.on_wait = [waits[-1]]
                    changed = True
                new_list.append(inst)
            if changed:
                bb.instructions[:] = new_list
    return nc


def _build_bass(debug=False):
    import concourse.bass as bass
    import concourse.tile as tile
    from concourse import mybir

    f32 = mybir.dt.float32
    bf16 = mybir.dt.bfloat16
    EXP = mybir.ActivationFunctionType.Exp

    nc = bass.Bass()
    dbg = {}
    if debug:
        dbg["qt0"] = nc.dram_tensor("dbg_qt0", [128, N], f32, kind="ExternalOutput")
        dbg["kt0"] = nc.dram_tensor("dbg_kt0", [128, N], f32, kind="ExternalOutput")
        dbg["v0"] = nc.dram_tensor("dbg_v0", [QB, NT * 64], f32, kind="ExternalOutput")
        dbg["outT0"] = nc.dram_tensor("dbg_outT0", [64, N], f32, kind="ExternalOutput")
        dbg["recipT0"] = nc.dram_tensor("dbg_recipT0", [QB, NT], f32, kind="ExternalOutput")
        dbg["xt"] = nc.dram_tensor("dbg_xt", [128, 4 * N], f32, kind="ExternalOutput")
    x_d = nc.dram_tensor("x", [N, D], f32, kind="ExternalInput")
    wq_d = nc.dram_tensor("wq", [D, 256], f32, kind="ExternalInput")
    wk_d = nc.dram_tensor("wk", [D, 256], f32, kind="ExternalInput")
    wv_d = nc.dram_tensor("wv", [D, 256], f32, kind="ExternalInput")
    bq_d = nc.dram_tensor("bq", [256], f32, kind="ExternalInput")
    bk_d = nc.dram_tensor("bk", [256], f32, kind="ExternalInput")
    bv_d = nc.dram_tensor("bv", [256], f32, kind="ExternalInput")
    mask_d = nc.dram_tensor("maskadd", [QB, QB], f32, kind="ExternalInput")
    id_d = nc.dram_tensor("ident", [128, 128], f32, kind="ExternalInput")
    attn_o = nc.dram_tensor("attn_o", [HPC, N, N], f32, kind="ExternalOutput")
    out_o = nc.dram_tensor("out_o", [N, 256], f32, kind="ExternalOutput")

    with tile.TileContext(nc) as tc:
        with contextlib.ExitStack() as ctx:
            cst = ctx.enter_context(tc.tile_pool(name="cst", bufs=1))
            qkt = ctx.enter_context(tc.tile_pool(name="qkt", bufs=1))
            vpool = ctx.enter_context(tc.tile_pool(name="vpool", bufs=1))
            # Shared PSUM pool: tag "a" 2 banks x2, "b" 2 banks x1,
            # "ot" 1 bank x2 -> 8 banks exactly.
            ps = ctx.enter_context(tc.tile_pool(name="ps", bufs=2, space="PSUM"))
            ps2 = ctx.enter_context(tc.tile_pool(name="ps2", bufs=1, space="PSUM"))
            psb = ctx.enter_context(tc.tile_pool(name="psb", bufs=1, space="PSUM"))

            phase01 = ctx.enter_context(contextlib.ExitStack())
            wpool = phase01.enter_context(tc.tile_pool(name="wpool", bufs=1))
            xtp = phase01.enter_context(tc.tile_pool(name="xtp", bufs=1))
            xin = phase01.enter_context(tc.tile_pool(name="xin", bufs=3))

            # ---- constants ----
            mask_sb = cst.tile([QB, QB], f32, tag="mask")
            nc.sync.dma_start(out=mask_sb[:], in_=mask_d[:])
            id_sb = cst.tile([128, 128], f32, tag="ident")
            nc.sync.dma_start(out=id_sb[:], in_=id_d[:])
            ones_sb = cst.tile([1, QB], bf16, tag="ones")
            nc.vector.memset(ones_sb[:], 1.0)
            bqp = cst.tile([128, 2], f32, tag="bqp")
            bkp = cst.tile([128, 2], f32, tag="bkp")
            nc.sync.dma_start(out=bqp[:], in_=bq_d.rearrange("(p d) -> d p", p=2))
            nc.sync.dma_start(out=bkp[:], in_=bk_d.rearrange("(p d) -> d p", p=2))
            bvr = cst.tile([1, 256], bf16, tag="bvr")
            nc.gpsimd.dma_start(out=bvr[:], in_=bv_d.rearrange("(o c) -> o c", o=1))

            # ---- weights ----
            w_sb = {}
            for nm, wd in (("q", wq_d), ("k", wk_d), ("v", wv_d)):
                w = wpool.tile([128, 4 * 256], bf16, tag=f"w{nm}")
                for di in range(4):
                    nc.gpsimd.dma_start(
                        out=w[:, di * 256 : (di + 1) * 256],
                        in_=wd[di * 128 : (di + 1) * 128, :],
                    )
                w_sb[nm] = w

            # ---- x -> xT  (xT[:, di*N + t*QB + j] = x[t*QB + j, di*128 + p]) ----
            xT = xtp.tile([128, 4 * N], bf16, tag="xT")
            for t in range(NT):
                xt_in = xin.tile([QB, D], f32)
                nc.sync.dma_start(out=xt_in[:], in_=x_d[t * QB : (t + 1) * QB, :])
                for di in range(4):
                    pt = ps.tile([128, QB], f32, tag="a")
                    nc.tensor.transpose(
                        pt[:], xt_in[:, di * 128 : (di + 1) * 128], id_sb[:QB, :QB]
                    )
                    nc.vector.tensor_copy(
                        xT[:, di * N + t * QB : di * N + (t + 1) * QB], pt[:]
                    )

            # ---- projections: QT/KT per pair [128, N] (two heads stacked) ----
            qt_sb = []
            kt_sb = []
            for p in range(2):
                qt = qkt.tile([128, N], bf16, tag=f"qt{p}")
                kt = qkt.tile([128, N], bf16, tag=f"kt{p}")
                for c in range(6):
                    cs = slice(c * 500, (c + 1) * 500)
                    for dst, w, bias in ((qt, w_sb["q"], bqp), (kt, w_sb["k"], bkp)):
                        psq = ps.tile([128, 500], f32, tag="a")
                        for di in range(4):
                            nc.tensor.matmul(
                                psq[:],
                                w[:, di * 256 + p * 128 : di * 256 + (p + 1) * 128],
                                xT[:, di * N + c * 500 : di * N + (c + 1) * 500],
                                start=(di == 0),
                                stop=(di == 3),
                            )
                        nc.vector.tensor_scalar_add(dst[:, cs], psq[:], bias[:, p : p + 1])
                qt_sb.append(qt)
                kt_sb.append(kt)

            # ---- V per head [QB, 24*65]; col 64 of each 65-block stays 1.0 ----
            v_sb = []
            for h in range(HPC):
                v = vpool.tile([QB, NT * 64], bf16, tag=f"v{h}")
                v_sb.append(v)
            for p in range(2):
                for t in range(NT):
                    psv = ps2.tile([QB, 128], f32, tag="sa")
                    for di in range(4):
                        nc.tensor.matmul(
                            psv[:],
                            xT[:, di * N + t * QB : di * N + (t + 1) * QB],
                            w_sb["v"][:, di * 256 + p * 128 : di * 256 + (p + 1) * 128],
                            start=(di == 0),
                            stop=False,
                        )
                    nc.tensor.matmul(
                        psv[:],
                        ones_sb[:1, :QB],
                        bvr[:1, p * 128 : (p + 1) * 128],
                        start=False,
                        stop=True,
                    )
                    for hh in range(2):
                        h = p * 2 + hh
                        nc.vector.tensor_scalar_max(
                            v_sb[h][:, t * 64 : (t + 1) * 64],
                            psv[:, hh * 64 : (hh + 1) * 64],
                            0.0,
                        )

            if debug:
                nc.sync.dma_start(out=dbg["xt"][:], in_=xT[:])
            phase01.close()

            phase2 = ctx.enter_context(contextlib.ExitStack())
            otp = phase2.enter_context(tc.tile_pool(name="otp", bufs=2))
            rcp = phase2.enter_context(tc.tile_pool(name="rcp", bufs=2))
            etp = phase2.enter_context(tc.tile_pool(name="etp", bufs=4))
            attp = phase2.enter_context(tc.tile_pool(name="attp", bufs=4))
            obp = phase2.enter_context(tc.tile_pool(name="obp", bufs=3))
            accp = phase2.enter_context(tc.tile_pool(name="accp", bufs=4))
            osg = phase2.enter_context(tc.tile_pool(name="osg", bufs=1))
            ostage = osg.tile([QB, NT * 256], f32, tag="ostage")

            # ---- per head: interleaved PV + attn ----
            def attn_step(h, p, hb, qt, kt, recipT, t):
                att = attp.tile([QB, N], f32, tag="att")
                acc = accp.tile([QB, 3], f32, tag="acc")
                for g2 in range(3):
                    sa = ps2.tile([QB, 1024], f32, tag="sa")
                    for c in range(2):
                        nc.tensor.matmul(
                            sa[:, c * 512 : c * 512 + 500],
                            qt[hb : hb + 64, t * QB : (t + 1) * QB],
                            kt[hb : hb + 64, g2 * 1000 + c * 500 : g2 * 1000 + (c + 1) * 500],
                        )
                    if t // 8 == g2:
                        off = t * QB - g2 * 1000
                        ccol = off if off < 500 else off + 12
                        nc.vector.tensor_add(
                            sa[:, ccol : ccol + QB], sa[:, ccol : ccol + QB], mask_sb[:]
                        )
                    nc.scalar.activation(
                        att[:, g2 * 1000 : (g2 + 1) * 1000].rearrange("p (c w) -> p c w", w=500),
                        sa[:].rearrange("p (c w) -> p c w", w=512)[:, :, 0:500],
                        EXP,
                        scale=0.125,
                        accum_out=acc[:, g2 : g2 + 1],
                    )
                nc.vector.tensor_add(acc[:, 0:1], acc[:, 0:1], acc[:, 1:2])
                nc.vector.tensor_add(acc[:, 0:1], acc[:, 0:1], acc[:, 2:3])
                nc.vector.reciprocal(recipT[:, t : t + 1], acc[:, 0:1])
                nc.vector.tensor_scalar_mul(att[:], att[:], recipT[:, t : t + 1])
                eng = nc.gpsimd if t % 2 == 0 else nc.sync
                eng.dma_start(out=attn_o[h, t * QB : (t + 1) * QB, :], in_=att[:])

            for h in range(HPC):
                p, hb = h // 2, (h % 2) * 64
                qt, kt, v = qt_sb[p], kt_sb[p], v_sb[h]

                outT = otp.tile([64, N], f32)
                recipT = rcp.tile([QB, NT], f32)
                for g in range(3):
                    bps = psb.tile([64, 1024], f32, tag="b")
                    for t in range(NT):
                        aps = ps.tile([QB, 1024], f32, tag="a")
                        for c in range(2):
                            nc.tensor.matmul(
                                aps[:, c * 512 : c * 512 + 500],
                                kt[hb : hb + 64, t * QB : (t + 1) * QB],
                                qt[hb : hb + 64, g * 1000 + c * 500 : g * 1000 + (c + 1) * 500],
                            )
                        if t // 8 == g:
                            off = t * QB - g * 1000
                            ccol = off if off < 500 else off + 12
                            nc.vector.tensor_add(
                                aps[:, ccol : ccol + QB], aps[:, ccol : ccol + QB], mask_sb[:]
                            )
                        et = etp.tile([QB, 1000], bf16)
                        nc.scalar.activation(
                            et[:].rearrange("p (c w) -> p c w", w=500),
                            aps[:].rearrange("p (c w) -> p c w", w=512)[:, :, 0:500],
                            EXP,
                            scale=0.125,
                        )
                        for c in range(2):
                            nc.tensor.matmul(
                                bps[:, c * 512 : c * 512 + 500],
                                v[:, t * 64 : (t + 1) * 64],
                                et[:, c * 500 : (c + 1) * 500],
                                start=(t == 0),
                                stop=(t == NT - 1),
                            )
                        if t % 3 == 2:
                            attn_step(h, p, hb, qt, kt, recipT, g * 8 + t // 3)
                    nc.vector.tensor_copy(
                        outT[:, g * 1000 : (g + 1) * 1000].rearrange("p (c w) -> p c w", w=500),
                        bps[:].rearrange("p (c w) -> p c w", w=512)[:, :, 0:500],
                    )

                # out epilogue: transpose out^T blocks, scale rows by recip
                for t in range(NT):
                    ot = psb.tile([QB, 64], f32, tag="b")
                    nc.tensor.transpose(
                        ot[:], outT[:, t * QB : (t + 1) * QB], id_sb[:64, :64]
                    )
                    nc.vector.tensor_scalar_mul(
                        ostage[:, t * 256 + h * 64 : t * 256 + (h + 1) * 64],
                        ot[:],
                        recipT[:, t : t + 1],
                    )

                if debug and h == 0:
                    nc.sync.dma_start(out=dbg["qt0"][:], in_=qt_sb[0][:])
                    nc.sync.dma_start(out=dbg["kt0"][:], in_=kt_sb[0][:])
                    nc.sync.dma_start(out=dbg["v0"][:], in_=v_sb[0][:])
                    nc.sync.dma_start(out=dbg["outT0"][:], in_=outT[:])
                    nc.sync.dma_start(out=dbg["recipT0"][:], in_=recipT[:])

            # ---- final out DMA ----
            nc.gpsimd.dma_start(
                out=out_o.rearrange("(t p) c -> p t c", p=QB),
                in_=ostage[:].rearrange("p (t c) -> p t c", c=256),
            )

    _fix_multiwait(nc)
    return nc


_CACHE = {}
TRACE = False
TRACE_KWARGS = {}
LAST_RESULT = [None]


def _get_nc():
    if "nc" not in _CACHE:
        _CACHE["nc"] = _build_bass()
    return _CACHE["nc"]


def _mask_np():
    block = np.kron(np.eye(5, dtype=np.float32), np.ones((J, J), np.float32))
    return (NEGM * (block - np.eye(QB, dtype=np.float32))).astype(np.float32)


def kernel(x, Wq, bq, Wk, bk, Wv, bv):
    _install_profile_hook()
    from concourse.bass_utils import run_bass_kernel_spmd
    from concourse import bass_utils

    bass_utils.upload_artifacts = lambda tmpdir: f"local://{tmpdir}"

    x = np.asarray(x, dtype=np.float32)
    Wq, Wk, Wv = (np.asarray(a, np.float32) for a in (Wq, Wk, Wv))
    bq, bk, bv = (np.asarray(a, np.float32) for a in (bq, bk, bv))

    mask = _mask_np()
    ident = np.eye(128, dtype=np.float32)

    in_maps = []
    for c in range(8):
        b, hp = c // 2, c % 2
        cols = slice(hp * 256, (hp + 1) * 256)
        in_maps.append(
            {
                "x": np.ascontiguousarray(x[b]),
                "wq": np.ascontiguousarray(Wq[:, cols]),
                "wk": np.ascontiguousarray(Wk[:, cols]),
                "wv": np.ascontiguousarray(Wv[:, cols]),
                "bq": np.ascontiguousarray(bq[cols]),
                "bk": np.ascontiguousarray(bk[cols]),
                "bv": np.ascontiguousarray(bv[cols]),
                "maskadd": mask,
                "ident": ident,
            }
        )

    nc = _get_nc()
    res = run_bass_kernel_spmd(
        nc, in_maps, list(range(8)), trace=TRACE, **TRACE_KWARGS
    )
    LAST_RESULT[0] = res

    out = np.empty((B, N, 512), np.float32)
    attn = np.empty((B, H, N, N), np.float32)
    for c in range(8):
        b, hp = c // 2, c % 2
        out[b, :, hp * 256 : (hp + 1) * 256] = res.results[c]["out_o"]
        attn[b, hp * 4 : (hp + 1) * 4] = res.results[c]["attn_o"]
    return out, attn
